# revision 35
# baseline (speedup 1.0000x reference)
"""Trainium2 Bass kernel: cosine-similarity softmin retrieval (DSDM).

reference:  qn = q/||q||; an = a/||a||; sims = qn @ an^T            [B, N]
            w = softmax(10*sims) over N  (softmin of (1-sims)/0.1)
            out = (w @ A)                                           [B, D]

Strategy (8 NeuronCores, flash-attention-style split over N):
  - addresses [200000, 512] sharded row-wise, 25000 rows/core.
  - each core streams its shard ONCE, cast fp32->fp8e4 during the load DMA
    (HBM reads stay fp32: 51.2 MB/core, the memory roofline).
  - per quad of 4 row-tiles (software-pipelined across iterations):
      * A^T via 16 PE transposes (fp8 writes 16-bit granules -> step-2
        PSUM layout) -> one dense uint16 DVE copy to SBUF
      * s_raw [64b, 512n] = qn^T(bf16) x A^T(fp8, step-2 view): 4 wide
        N=512 mixed-dtype matmuls accumulated in PSUM
      * s_sc = s_raw * (10/||a||) on DVE (inv broadcast via SBUF scratch)
      * w = Exp(s_sc - 10) bf16 on ACT, accum_out -> per-quad wsum column
        (fixed shift: cos<=1 so logit-10 <= 0; no running max needed)
      * w^T via 4 PE transposes (bf16) + DVE copy
      * acc [64, 512] += w^T.T(bf16) @ A(fp8) in PSUM across all tiles
  - row norms ss = sum(a^2) split DVE (affine_mul_reduce) / ACT (Square),
    spread tile-by-tile across quads to keep the FIFO engines responsive
  - 10/||a|| = exp(-0.5*ln(ss + eps) + ln10) on ACT (one table set)
  - host: out = sum_c acc_c / sum_c l_c   (gather/unshard + tiny divide)

Padding: per-core row count 25000 = 195*128 + 40; the last tile's 88 pad
rows are zeroed; their s_sc is 0 so they get weight exp(-10), subtracted
exactly on the host.
"""

import math
import os
from collections import OrderedDict

import numpy as np

import concourse.bass as bass
import concourse.tile as tile
from concourse import bacc, mybir
from concourse.bass_utils import run_bass_kernel_spmd
from concourse.masks import make_identity

DT = mybir.dt
AF = mybir.ActivationFunctionType

B = 64
D = 512
N_FULL = 200000
NCORES = 8
NPC = N_FULL // NCORES  # 25000
P = 128
LN10 = math.log(10.0)

G = int(os.environ.get("KERNEL_G", "28"))  # tiles per DMA slab (4 | G)
FP8 = os.environ.get("KERNEL_FP8", "1") == "1"
NORM_DVE_OF8 = int(os.environ.get("KERNEL_NORM_DVE_OF8", "3"))  # tiles/8 on DVE
SLAB_BUFS = int(os.environ.get("KERNEL_SLAB_BUFS", "4"))
NORM_AHEAD = int(os.environ.get("KERNEL_NORM_AHEAD", "5"))  # quads of norm lead
BACK_DEPTH = int(os.environ.get("KERNEL_BACK_DEPTH", "3"))

LAST_RESULTS = None  # test harness reads exec_time_ns from here


def _patch_act_tables():
    """Prefer the combined natural_log_exp set so Ln/Exp/Square/Copy share
    one ACT table load instead of thrashing 2 loads per slab (~2.7us each)."""
    if getattr(bacc.get_activation_tables, "_patched", False):
        return
    orig = bacc.get_activation_tables

    keep = {AF.Ln, AF.Exp, AF.Square}

    def patched(arch):
        tabs = orig(arch)
        out = OrderedDict()
        for k, fns in tabs.items():
            if k == "natural_log_exp_and_others":
                out[k] = fns
            else:
                out[k] = {f for f in fns if f not in keep}
        return out

    patched._patched = True
    bacc.get_activation_tables = patched


def _build(npc=NPC):
    _patch_act_tables()
    ntiles = (npc + P - 1) // P
    assert ntiles % 4 == 0
    nquads = ntiles // 4
    g = G
    nslabs = (ntiles + g - 1) // g
    real_last = npc - (ntiles - 1) * P  # rows in final tile
    adt = DT.float8e4 if FP8 else DT.bfloat16

    nc = bacc.Bacc("TRN2")
    q_d = nc.dram_tensor("query", [B, D], DT.float32, kind="ExternalInput")
    a_d = nc.dram_tensor("addresses", [npc, D], DT.float32, kind="ExternalInput")
    acc_d = nc.dram_tensor("acc", [B, D], DT.float32, kind="ExternalOutput")
    lsum_d = nc.dram_tensor("lsum", [B, nquads], DT.float32, kind="ExternalOutput")

    with tile.TileContext(nc) as tc:
        with (
            tc.tile_pool(name="const", bufs=1) as const,
            tc.tile_pool(name="slab", bufs=SLAB_BUFS) as slab_pool,
            tc.tile_pool(name="at", bufs=2) as at_pool,
            tc.tile_pool(name="wt", bufs=5) as wt_pool,
            tc.tile_pool(name="ssc", bufs=2) as ssc_pool,
            tc.tile_pool(name="small", bufs=4) as small,
            tc.tile_pool(name="ps_at", bufs=2, space="PSUM") as ps_at,
            tc.tile_pool(name="ps_s", bufs=2, space="PSUM") as ps_s,
            tc.tile_pool(name="ps_wt", bufs=1, space="PSUM") as ps_wt,
            tc.tile_pool(name="ps_acc", bufs=1, space="PSUM") as ps_acc,
            tc.tile_pool(name="dram", bufs=1, space="DRAM") as dram_pool,
        ):
            ident = const.tile([P, P], adt)
            make_identity(nc, ident)
            identb = const.tile([B, B], DT.bfloat16)
            make_identity(nc, identb)
            bias_main = const.tile([B, 1], DT.float32)
            nc.vector.memset(bias_main, -10.0)
            eps12 = const.tile([P, 1], DT.float32)
            nc.vector.memset(eps12, 1e-12)
            ln10b = const.tile([P, 1], DT.float32)
            nc.vector.memset(ln10b, LN10)
            identf = const.tile([P, P], DT.float32)
            make_identity(nc, identf)
            wsums = const.tile([B, nquads], DT.float32)

            def wt_tile():
                # single shared PSUM bank: w^T transposes, qn prep, ivt
                return ps_wt.tile([P, 4, B], DT.bfloat16, tag="wt", name="wtps")

            # ---- query preprocessing: qn^T bf16 chunks [128d, 4c, 64b] ----
            q_sb = const.tile([B, D], DT.float32)
            nc.sync.dma_start(out=q_sb, in_=q_d[:, :])
            qsq = const.tile([B, D], DT.float32)
            ssq = const.tile([B, 1], DT.float32)
            nc.scalar.activation(qsq, q_sb, AF.Square, accum_out=ssq)
            lnq = const.tile([B, 1], DT.float32)
            nc.scalar.activation(lnq, ssq, AF.Ln, bias=eps12[:B])
            invq = const.tile([B, 1], DT.float32)
            nc.scalar.activation(invq, lnq, AF.Exp, scale=-0.5)
            qn = const.tile([B, D], DT.bfloat16)
            nc.vector.tensor_scalar_mul(out=qn, in0=q_sb, scalar1=invq)
            qnT = const.tile([P, 4, B], DT.bfloat16)
            for c in range(4):
                qt_ps = wt_tile()
                nc.tensor.transpose(qt_ps[:, 0, :], qn[:, c * P:(c + 1) * P],
                                    identb)
                nc.scalar.copy(qnT[:, c, :], qt_ps[:, 0, :])

            # ---- main streaming loop ----
            # A-stream segments: the first two slabs arrive in 4-tile chunks
            # (fast pipeline fill), the rest as big G-tile DMAs (bandwidth).
            segs = []
            t = 0
            while t < ntiles:
                step = 4 if t < g else g
                segs.append((t, min(t + step, ntiles)))
                t += step

            def seg_idx(gt):
                if gt < g:
                    return gt // 4
                return g // 4 + (gt - g) // g

            # flattened 10/||a|| per tile, DRAM scratch (single-partition SBUF
            # sources serialize on one AXI port; DRAM broadcasts don't).
            # One tile per 14-tile piece: Tile tracks DRAM deps per-tensor, so
            # a shared tensor would stall inv_bc reads on unrelated writes.
            FIN = 14
            npieces = (ntiles + FIN - 1) // FIN
            scrs = []
            for _k in range(npieces):
                scr_k = dram_pool.tile([1, FIN * P], DT.float32, name=f"scr{_k}")
                scrs.append(scr_k)
            acc_ps = ps_acc.tile([B, D], DT.float32)
            slab_tiles = {}
            slab_ss = {}
            norms_done = [0]  # tiles with norms emitted (in order)

            def ensure_seg(si):
                if si in slab_tiles:
                    return slab_tiles[si]
                t0, t1 = segs[si]
                gg = t1 - t0
                a_sl = slab_pool.tile([P, gg, D], adt, tag=f"a{gg}",
                                      name=f"asl{gg}",
                                      bufs=(16 if gg == 4 else SLAB_BUFS))
                last_seg = t1 == ntiles
                if not last_seg or real_last == P:
                    nc.gpsimd.dma_start(
                        out=a_sl,
                        in_=a_d[t0 * P:t1 * P, :].rearrange(
                            "(t p) d -> p t d", p=P))
                else:
                    for t in range(gg - 1):
                        r0 = (t0 + t) * P
                        nc.gpsimd.dma_start(out=a_sl[:, t, :], in_=a_d[r0:r0 + P, :])
                    nc.gpsimd.memset(a_sl[:, gg - 1, :], 0)
                    nc.gpsimd.dma_start(
                        out=a_sl[:real_last, gg - 1, :],
                        in_=a_d[(ntiles - 1) * P:npc, :])
                slab_tiles[si] = a_sl
                return a_sl

            def a_tile(gt):
                si = seg_idx(gt)
                return ensure_seg(si)[:, gt - segs[si][0], :]

            def norm_tiles_upto(gt_end):
                """Emit per-tile norms (spread across quads) and, at each
                FIN-piece's last tile, the inv finalize + scr flatten."""
                while norms_done[0] < min(gt_end, ntiles):
                    gt = norms_done[0]
                    k, t = divmod(gt, FIN)
                    ff = min(FIN, ntiles - k * FIN)
                    if t == 0:
                        ss_new = small.tile([P, FIN], DT.float32, tag="ss")
                        slab_ss[k] = ss_new
                    ss = slab_ss[k]
                    sq = small.tile([P, D], DT.bfloat16, tag="sq")
                    a_t = a_tile(gt)
                    if (gt % 8) < NORM_DVE_OF8:
                        nc.vector.affine_mul_reduce(
                            out=sq, accum_out=ss[:, t:t + 1],
                            in0=a_t, in1=a_t, scale=1.0, bias=0.0)
                    else:
                        nc.scalar.activation(sq, a_t, AF.Square,
                                             accum_out=ss[:, t:t + 1])
                    if t == ff - 1:
                        lns = small.tile([P, FIN], DT.float32, tag="lns")
                        nc.scalar.activation(lns[:, :ff], ss[:, :ff],
                                             AF.Ln, bias=eps12)
                        inv = small.tile([P, FIN], DT.float32, tag="inv")
                        nc.scalar.activation(inv[:, :ff], lns[:, :ff], AF.Exp,
                                             scale=-0.5, bias=ln10b)
                        # transpose inv -> [ff, 128] (reusing the wt PSUM bank
                        # through an fp32 view), flatten into scr piece k
                        iv_ps = wt_tile().bitcast(DT.float32).rearrange(
                            "p a b -> p (a b)")
                        nc.tensor.transpose(iv_ps[:ff, :P], inv[:, :ff], identf)
                        ivt = small.tile([FIN, P], DT.float32, tag="ivt_sb")
                        nc.vector.tensor_copy(ivt[:ff], iv_ps[:ff, :P])
                        scr_k = scrs[k]
                        # ACT-issued HWDGE: keeps the SP queue free so
                        # inv_bc reads never block behind this write's wait
                        nc.scalar.dma_start(
                            out=bass.AP(tensor=scr_k.tensor,
                                        offset=scr_k.offset,
                                        ap=[[P, ff], [1, P]]),
                            in_=ivt[:ff])
                    norms_done[0] += 1

            def front_a(q):
                # A^T for the quad: [128d, 4c, 512n]. fp8 transposes write
                # 16-bit granules (value in the low byte): step-2 PSUM tile,
                # dense uint16 copy out, fp8 step-2 view for the matmul.
                if FP8:
                    at_ps = ps_at.tile([P, 4, 4 * P, 2], adt)
                    for t in range(4):
                        a_t = a_tile(4 * q + t)
                        for c in range(4):
                            nc.tensor.transpose(
                                at_ps[:, c, t * P:(t + 1) * P, 0],
                                a_t[:, c * P:(c + 1) * P], ident)
                    at_sb = at_pool.tile([P, 4, 4 * P], DT.uint16)
                    nc.vector.tensor_copy(at_sb,
                                          at_ps.bitcast(DT.uint16)[:, :, :, 0])
                else:
                    at_ps = ps_at.tile([P, 4, 4 * P], adt)
                    for t in range(4):
                        a_t = a_tile(4 * q + t)
                        for c in range(4):
                            nc.tensor.transpose(
                                at_ps[:, c, t * P:(t + 1) * P],
                                a_t[:, c * P:(c + 1) * P], ident)
                    at_sb = at_pool.tile([P, 4, 4 * P], adt)
                    nc.vector.tensor_copy(at_sb, at_ps)
                # inv broadcast [64b, 512n] from the per-piece DRAM scratch
                inv_bc = ssc_pool.tile([B, 4 * P], DT.float32, tag="inv_bc")
                v0 = 4 * q * P  # first value index of this quad
                off = 0
                while off < 4 * P:
                    k = (v0 + off) // (FIN * P)
                    k_end = (k + 1) * FIN * P
                    span = min(4 * P - off, k_end - (v0 + off))
                    scr_k = scrs[k]
                    nc.sync.dma_start(
                        out=inv_bc[:, off:off + span],
                        in_=bass.AP(tensor=scr_k.tensor,
                                    offset=scr_k.offset
                                    + (v0 + off) - k * FIN * P,
                                    ap=[[0, B], [1, span]]))
                    off += span
                return at_sb, inv_bc

            def front_b(q, at_sb, inv_bc):
                # sims: 4 wide mixed-dtype matmuls accumulating over d-chunks
                s_ps = ps_s.tile([B, 4 * P], DT.float32, tag="s")
                if FP8:
                    at8 = at_sb.bitcast(DT.float8e4).rearrange(
                        "p k (n two) -> p k n two", two=2)
                    for c in range(4):
                        nc.tensor.matmul(
                            s_ps, lhsT=qnT[:, c, :], rhs=at8[:, c, :, 0],
                            start=(c == 0), stop=(c == 3))
                else:
                    for c in range(4):
                        nc.tensor.matmul(
                            s_ps, lhsT=qnT[:, c, :], rhs=at_sb[:, c, :],
                            start=(c == 0), stop=(c == 3))
                s_sc = ssc_pool.tile([B, 4 * P], DT.float32, tag="s_sc")
                nc.vector.tensor_mul(s_sc, s_ps, inv_bc)
                w_q = wt_pool.tile([B, 4 * P], DT.bfloat16, tag="w_q")
                nc.scalar.activation(w_q, s_sc, AF.Exp, bias=bias_main,
                                     accum_out=wsums[:, q:q + 1])
                return w_q

            def stage_back(q, w_q):
                wt_ps = wt_tile()
                for t in range(4):
                    nc.tensor.transpose(
                        wt_ps[:, t, :], w_q[:, t * P:(t + 1) * P], identb)
                wt_sb = wt_pool.tile([P, 4, B], DT.bfloat16, tag="wt_sb")
                nc.vector.tensor_copy(wt_sb, wt_ps)
                for t in range(4):
                    gt = 4 * q + t
                    nc.tensor.matmul(
                        acc_ps, lhsT=wt_sb[:, t, :], rhs=a_tile(gt),
                        start=(gt == 0), stop=(gt == ntiles - 1))

            # Software pipeline, skewed so every cross-engine dependency has
            # a full iteration of slack (no FIFO priority inversions):
            #   iter i emits: front_a(i) | back(i-BD) | norms | front_b(i-1)
            norm_tiles_upto(4 * NORM_AHEAD)
            fr = {}
            wq = {}
            BD = BACK_DEPTH
            for q in range(nquads + BD):
                if q < nquads:
                    t_ahead = min(4 * (q + NORM_AHEAD) + g, ntiles - 1)
                    ensure_seg(seg_idx(t_ahead))
                    fr[q] = front_a(q)
                if q - BD >= 0:
                    stage_back(q - BD, wq.pop(q - BD))
                if q < nquads:
                    norm_tiles_upto(4 * (q + 1 + NORM_AHEAD))
                if q - 1 >= 0 and q - 1 < nquads:
                    wq[q - 1] = front_b(q - 1, *fr.pop(q - 1))

            # ---- epilogue: writeback ----
            acc_sb = const.tile([B, D], DT.float32)
            nc.scalar.copy(acc_sb, acc_ps)
            nc.sync.dma_start(out=acc_d[:, :], in_=acc_sb)
            nc.sync.dma_start(out=lsum_d[:, :], in_=wsums)

    nc.finalize()
    return nc


_NC_CACHE = {}


def _get_nc(npc=NPC):
    if npc not in _NC_CACHE:
        _NC_CACHE[npc] = _build(npc)
    return _NC_CACHE[npc]


def kernel(query, addresses):
    global LAST_RESULTS
    query = np.ascontiguousarray(np.asarray(query), dtype=np.float32)
    addresses = np.ascontiguousarray(np.asarray(addresses), dtype=np.float32)
    n = addresses.shape[0]
    npc = n // NCORES
    assert npc * NCORES == n
    nc = _get_nc(npc)
    in_maps = [
        {"query": query, "addresses": addresses[c * npc:(c + 1) * npc]}
        for c in range(NCORES)
    ]
    res = run_bass_kernel_spmd(nc, in_maps, core_ids=list(range(NCORES)))
    LAST_RESULTS = res
    acc = np.zeros((B, D), np.float64)
    l = np.zeros((B, 1), np.float64)
    ntiles = (npc + P - 1) // P
    n_pad = ntiles * P - npc  # zero rows in the padded last tile
    for r in res.results:
        acc += r["acc"].astype(np.float64)
        l += r["lsum"].astype(np.float64).sum(axis=1, keepdims=True)
        if n_pad:
            # each pad row contributes exactly exp(0*scale - 10)
            l -= n_pad * math.exp(-10.0)
    return (acc / l).astype(np.float32)
